# revision 1
# baseline (speedup 1.0000x reference)
"""Equivariant PQ-layer conv kernel for 8x TRN2 NeuronCores.

Strategy: the layer is a 3D conv (SAME, 5^3 taps) with an assembled
(320, 320, 125) kernel over a (320, 8^3) input. The host assembles the
conv kernel from the geometry/weight inputs (cheap: ~0.2 GFLOP vs the
13 GFLOP conv) and shards the 125 taps across the 8 cores. Each core
gets 8 TAP-PAIR slots (2 taps = 640 contraction rows = exactly 5x128
chunks, so every matmul runs at full K=128). Per pair: 5 contraction
chunks x 3 output chunks of N=512-voxel matmuls (fp16 operands, fp32
PSUM accumulate across all pairs). The host sums the 8 partial outputs
(the unshard for a contraction-parallel split) and adds the bias.
"""
import numpy as np

C0, C1 = 8, 4
K = 5
G = 8
EPS = 1e-6
R_MAX = 5.5
DIM = C0 + 3 * C1          # 20
Q = 16
P = 8
NCH = DIM * Q              # 320
NV = P * P * P             # 512
K3 = K ** 3                # 125
NCORES = 8
PAIRS = 8                  # tap-pair slots per core: 8*8*2 = 128 >= 125 taps
NCC = 5                    # contraction chunks per pair: 2*320/128
OCH = [(0, 128), (128, 128), (256, 64)]

LAST = None                # BassKernelResults of the most recent run (for test harness)
_PROGRAM = None


def _levi_civita():
    e = np.zeros((3, 3, 3), np.float32)
    e[0, 1, 2] = e[1, 2, 0] = e[2, 0, 1] = 1.0
    e[0, 2, 1] = e[2, 1, 0] = e[1, 0, 2] = -1.0
    return e


def _assemble_kern(q_in, q_out, w_ss, w_vs, w_sv, w_vv0, w_vv1):
    """Mirror of the reference kernel assembly, in f32 numpy. -> (320, 320, 125)."""
    offs = np.arange(K, dtype=np.float32) - (K - 1) / 2.0
    oz, oy, ox = np.meshgrid(offs, offs, offs, indexing='ij')
    p_off = np.stack([oz, oy, ox], -1).reshape(-1, 3)
    v = p_off[None, None] - (q_out[:, None, None] - q_in[None, :, None])
    r = np.linalg.norm(v, axis=-1)
    u = np.where(r[..., None] > EPS, v / np.maximum(r, EPS)[..., None], 0.0).astype(np.float32)
    centers = np.linspace(0.0, R_MAX, G).astype(np.float32)
    sigma = R_MAX / (G - 1)
    R = np.exp(-0.5 * ((r[..., None] - centers) / sigma) ** 2).astype(np.float32)
    RY = R[..., None] * u[..., None, :]
    eye3 = np.eye(3, dtype=np.float32)
    eps3 = _levi_civita()
    K_ss = np.einsum('acg,pqkg->apcqk', w_ss, R, optimize=True)
    K_vs = np.einsum('acg,pqkgm->ampcqk', w_vs, RY, optimize=True)
    K_sv = np.einsum('acg,pqkgm->apcmqk', w_sv, RY, optimize=True)
    K_vv = (np.einsum('acg,pqkg,mn->ampcnqk', w_vv0, R, eye3, optimize=True)
            + np.float32(0.7071067811865476) *
            np.einsum('acg,pqkgm,imj->aipcjqk', w_vv1, RY, eps3, optimize=True))
    Qo, Qi = q_out.shape[0], q_in.shape[0]
    top = np.concatenate([K_ss, K_sv.reshape(C0, Qo, 3 * C1, Qi, K3)], axis=2)
    bot = np.concatenate([K_vs.reshape(3 * C1, Qo, C0, Qi, K3),
                          K_vv.reshape(3 * C1, Qo, 3 * C1, Qi, K3)], axis=2)
    kern = np.concatenate([top, bot], axis=0)
    return np.ascontiguousarray(kern.reshape(DIM * Qo, DIM * Qi, K3).astype(np.float32))


def _build_program():
    """One SPMD program: 8 tap-pair slots of (ker, xsh) -> partial conv output.

    Raw bass (no Tile): explicit engine blocks + standalone wait_ge
    instructions, one DMA-completion semaphore per pair slot.
    """
    global _PROGRAM
    if _PROGRAM is not None:
        return _PROGRAM
    from contextlib import ExitStack
    from concourse import bass, mybir

    nc = bass.Bass("TRN2", target_bir_lowering=False, debug=False,
                   enable_asserts=False, num_devices=NCORES)
    # Contraction chunks cc=0..4 packed side-by-side in the free dim.
    ker_d = nc.dram_tensor("ker", [PAIRS, 128, NCC * NCH], mybir.dt.float16,
                           kind="ExternalInput").ap()
    xsh_d = nc.dram_tensor("xsh", [PAIRS, 128, NCC * NV], mybir.dt.float16,
                           kind="ExternalInput").ap()
    # (128, 3*512): o-chunk oc lives in columns [oc*512, (oc+1)*512); host unpacks.
    out_d = nc.dram_tensor("out_part", [128, 3 * NV], mybir.dt.float32,
                           kind="ExternalOutput").ap()

    with ExitStack() as ctx:
        ktiles = [ctx.enter_context(nc.sbuf_tensor(f"kt{p}", [128, NCC * NCH], mybir.dt.float16))
                  for p in range(PAIRS)]
        xtiles = [ctx.enter_context(nc.sbuf_tensor(f"xt{p}", [128, NCC * NV], mybir.dt.float16))
                  for p in range(PAIRS)]
        otile = ctx.enter_context(nc.sbuf_tensor("otile", [128, 3 * NV], mybir.dt.float32))
        psum = [ctx.enter_context(nc.psum_tensor(f"psum{i}", [osz, NV], mybir.dt.float32))
                for i, (o0, osz) in enumerate(OCH)]
        dsemA = [ctx.enter_context(nc.semaphore(f"dsemA{p}")) for p in range(PAIRS)]
        dsemB = [ctx.enter_context(nc.semaphore(f"dsemB{p}")) for p in range(PAIRS)]
        psem = ctx.enter_context(nc.semaphore("psem"))
        vsem = ctx.enter_context(nc.semaphore("vsem"))
        osem = ctx.enter_context(nc.semaphore("osem"))
        block = ctx.enter_context(nc.Block())

        # ker loads on the SP HWDGE queue, xsh loads on the ACT HWDGE queue;
        # each pair's load is split in two halves (chunks 0-1 / 2-4) with
        # per-half sems so the PE starts on a pair after ~40% of its bytes.
        nA_k, nA_x = 2 * NCH, 2 * NV
        HALVES = (([0, 1], dsemA), ([2, 3, 4], dsemB))

        @block.sync
        def _(sync):
            for p in range(PAIRS):
                sync.dma_start(out=ktiles[p][:, :nA_k], in_=ker_d[p, :, :nA_k]).then_inc(dsemA[p], 16)
                sync.dma_start(out=ktiles[p][:, nA_k:], in_=ker_d[p, :, nA_k:]).then_inc(dsemB[p], 16)

        @block.scalar
        def _(scalar):
            for p in range(PAIRS):
                scalar.dma_start(out=xtiles[p][:, :nA_x], in_=xsh_d[p, :, :nA_x]).then_inc(dsemA[p], 16)
                scalar.dma_start(out=xtiles[p][:, nA_x:], in_=xsh_d[p, :, nA_x:]).then_inc(dsemB[p], 16)

        @block.tensor
        def _(tensor):
            for p in range(PAIRS):
                for ccs, sems in HALVES:
                    tensor.wait_ge(sems[p], 32)
                    for oc, (o0, osz) in enumerate(OCH):
                        for cc in ccs:
                            mm = tensor.matmul(
                                psum[oc][:, :],
                                ktiles[p][:, cc * NCH + o0:cc * NCH + o0 + osz],
                                xtiles[p][:, cc * NV:(cc + 1) * NV],
                                start=(p == 0 and cc == 0),
                                stop=(p == PAIRS - 1 and cc == NCC - 1))
                            if p == PAIRS - 1 and oc == len(OCH) - 1 and cc == NCC - 1:
                                mm.then_inc(psem, 1)

        @block.vector
        def _(vector):
            vector.wait_ge(psem, 1)
            for oc, (o0, osz) in enumerate(OCH):
                vector.tensor_copy(
                    otile[:osz, oc * NV:(oc + 1) * NV], psum[oc][:, :]
                ).then_inc(vsem, 1)

        @block.gpsimd
        def _(gpsimd):
            # store each o-chunk as soon as its PSUM->SBUF copy lands,
            # overlapping the store stream with the remaining copies
            for oc, (o0, osz) in enumerate(OCH):
                gpsimd.wait_ge(vsem, oc + 1)
                gpsimd.dma_start(out=out_d[:osz, oc * NV:(oc + 1) * NV],
                                 in_=otile[:osz, oc * NV:(oc + 1) * NV]).then_inc(osem, 16)
            gpsimd.wait_ge(osem, 16 * len(OCH))

    _PROGRAM = nc
    return nc


def kernel(x, q_in, q_out, w_ss, w_vs, w_sv, w_vv0, w_vv1, bias):
    global LAST
    from concourse.bass_utils import run_bass_kernel_spmd

    kern = _assemble_kern(np.asarray(q_in, np.float32), np.asarray(q_out, np.float32),
                          np.asarray(w_ss, np.float32), np.asarray(w_vs, np.float32),
                          np.asarray(w_sv, np.float32), np.asarray(w_vv0, np.float32),
                          np.asarray(w_vv1, np.float32))
    xr = np.asarray(x, np.float32).reshape(NCH, P, P, P)
    x_pad = np.zeros((NCH, P + 4, P + 4, P + 4), np.float32)
    x_pad[:, 2:10, 2:10, 2:10] = xr

    # Shifted input per tap (+1 zero slab for padding slots), fp16.
    xsh = np.zeros((K3 + 1, NCH, NV), np.float16)
    t = 0
    for dz in range(K):
        for dy in range(K):
            for dx in range(K):
                xsh[t] = x_pad[:, dz:dz + 8, dy:dy + 8, dx:dx + 8].reshape(NCH, NV)
                t += 1
    kerT = np.zeros((K3 + 1, NCH, NCH), np.float16)          # (tap, i, o)
    kerT[:K3] = kern.transpose(2, 1, 0)

    in_maps = []
    for c in range(NCORES):
        taps = list(range(c, K3, NCORES)) + [K3] * (2 * PAIRS)  # pad w/ zero slab
        taps = taps[:2 * PAIRS]
        ker_c = np.empty((PAIRS, 128, NCC * NCH), np.float16)
        xsh_c = np.empty((PAIRS, 128, NCC * NV), np.float16)
        for p in range(PAIRS):
            tA, tB = taps[2 * p], taps[2 * p + 1]
            kb = np.concatenate([kerT[tA], kerT[tB]], axis=0)    # (640, 320)
            xb = np.concatenate([xsh[tA], xsh[tB]], axis=0)      # (640, 512)
            ker_c[p] = kb.reshape(NCC, 128, NCH).transpose(1, 0, 2).reshape(128, NCC * NCH)
            xsh_c[p] = xb.reshape(NCC, 128, NV).transpose(1, 0, 2).reshape(128, NCC * NV)
        in_maps.append({"ker": ker_c, "xsh": xsh_c})

    nc = _build_program()
    res = run_bass_kernel_spmd(nc, in_maps, list(range(NCORES)))
    LAST = res

    out = np.zeros((NCH, NV), np.float32)
    for c in range(NCORES):
        arr = res.results[c]["out_part"]          # (128, 3*512) packed o-chunks
        for oc, (o0, osz) in enumerate(OCH):
            out[o0:o0 + osz] += arr[:osz, oc * NV:(oc + 1) * NV]
    out = out.reshape(1, DIM, Q, P, P, P).copy()
    out[:, :C0] += np.asarray(bias, np.float32).reshape(1, C0, 1, 1, 1, 1)
    return out



# revision 2
# speedup vs baseline: 1.2078x; 1.2078x over previous
"""Equivariant PQ-layer conv kernel for 8x TRN2 NeuronCores.

The layer is a 3D conv (SAME, 5^3 taps) with an assembled (320, 320, 125)
kernel over a (320, 8^3) input. The host assembles the conv kernel (cheap)
and shards the 125 taps across the 8 cores (16 tap slots per core, as 8
tap-pairs whose 640 contraction rows split into 5 exact 128-chunks).

Matmul arrangement ("transposed" vs the v1 baseline): PSUM partitions carry
voxels (512 = 4x128 chunks, no partition waste) and the free dim carries all
320 output channels (N=320 <= 512). Per pair: 5 K-chunks x 4 vox-chunks =
20 matmuls of N=320 -> 51200 charged PE rows/core vs 61440 for the
out-channels-on-partitions form (PSUM M=64 waste). lhsT (stationary) is the
shifted-input chunk [128 contraction rows, 128 voxels]; rhs (moving) is the
kernel chunk [128 contraction rows, 320 out channels].

The host pre-shifts x per tap (SPMD program must be identical across cores,
so per-core tap shifts must live in data, not in AP constants).
"""
import numpy as np

C0, C1 = 8, 4
K = 5
G = 8
EPS = 1e-6
R_MAX = 5.5
DIM = C0 + 3 * C1          # 20
Q = 16
P = 8
NCH = DIM * Q              # 320
NV = P * P * P             # 512
K3 = K ** 3                # 125
NCORES = 8
PAIRS = 8                  # tap-pair slots per core: 8*2*8 = 128 >= 125 taps
NCC = 5                    # contraction chunks per pair: 2*320/128
NM = 4                     # vox chunks (psum banks): 512/128

LAST = None                # BassKernelResults of the most recent run
_PROGRAM = None


def _levi_civita():
    e = np.zeros((3, 3, 3), np.float32)
    e[0, 1, 2] = e[1, 2, 0] = e[2, 0, 1] = 1.0
    e[0, 2, 1] = e[2, 1, 0] = e[1, 0, 2] = -1.0
    return e


def _assemble_kern(q_in, q_out, w_ss, w_vs, w_sv, w_vv0, w_vv1):
    """Mirror of the reference kernel assembly, in f32 numpy. -> (320, 320, 125)."""
    offs = np.arange(K, dtype=np.float32) - (K - 1) / 2.0
    oz, oy, ox = np.meshgrid(offs, offs, offs, indexing='ij')
    p_off = np.stack([oz, oy, ox], -1).reshape(-1, 3)
    v = p_off[None, None] - (q_out[:, None, None] - q_in[None, :, None])
    r = np.linalg.norm(v, axis=-1)
    u = np.where(r[..., None] > EPS, v / np.maximum(r, EPS)[..., None], 0.0).astype(np.float32)
    centers = np.linspace(0.0, R_MAX, G).astype(np.float32)
    sigma = R_MAX / (G - 1)
    R = np.exp(-0.5 * ((r[..., None] - centers) / sigma) ** 2).astype(np.float32)
    RY = R[..., None] * u[..., None, :]
    eye3 = np.eye(3, dtype=np.float32)
    eps3 = _levi_civita()
    K_ss = np.einsum('acg,pqkg->apcqk', w_ss, R, optimize=True)
    K_vs = np.einsum('acg,pqkgm->ampcqk', w_vs, RY, optimize=True)
    K_sv = np.einsum('acg,pqkgm->apcmqk', w_sv, RY, optimize=True)
    K_vv = (np.einsum('acg,pqkg,mn->ampcnqk', w_vv0, R, eye3, optimize=True)
            + np.float32(0.7071067811865476) *
            np.einsum('acg,pqkgm,imj->aipcjqk', w_vv1, RY, eps3, optimize=True))
    Qo, Qi = q_out.shape[0], q_in.shape[0]
    top = np.concatenate([K_ss, K_sv.reshape(C0, Qo, 3 * C1, Qi, K3)], axis=2)
    bot = np.concatenate([K_vs.reshape(3 * C1, Qo, C0, Qi, K3),
                          K_vv.reshape(3 * C1, Qo, 3 * C1, Qi, K3)], axis=2)
    kern = np.concatenate([top, bot], axis=0)
    return np.ascontiguousarray(kern.reshape(DIM * Qo, DIM * Qi, K3).astype(np.float32))


def _build_program():
    """SPMD program: 8 tap-pair slots of (kp, xs) -> partial conv output.

    Per pair p and contraction chunk c (5 chunks of 128 rows from the
    640-row tap pair): for vox chunk m: psum[m][128 vox, 320 out] +=
    xs[p][:, c*512+m*128 : +128].T @ kp[p][:, c*320 : +320].
    """
    global _PROGRAM
    if _PROGRAM is not None:
        return _PROGRAM
    from contextlib import ExitStack
    from concourse import bass, mybir

    nc = bass.Bass("TRN2", target_bir_lowering=False, debug=False,
                   enable_asserts=False, num_devices=NCORES)
    kp_d = nc.dram_tensor("kp", [PAIRS, 128, NCC * NCH], mybir.dt.float16,
                          kind="ExternalInput").ap()
    xs_d = nc.dram_tensor("xs", [PAIRS, 128, NCC * NV], mybir.dt.float16,
                          kind="ExternalInput").ap()
    out_d = nc.dram_tensor("out_part", [128, NM * NCH], mybir.dt.float32,
                           kind="ExternalOutput").ap()

    with ExitStack() as ctx:
        kpt = [ctx.enter_context(nc.sbuf_tensor(f"kpt{p}", [128, NCC * NCH], mybir.dt.float16))
               for p in range(PAIRS)]
        xst = [ctx.enter_context(nc.sbuf_tensor(f"xst{p}", [128, NCC * NV], mybir.dt.float16))
               for p in range(PAIRS)]
        ot = ctx.enter_context(nc.sbuf_tensor("ot", [128, NM * NCH], mybir.dt.float32))
        ps = [ctx.enter_context(nc.psum_tensor(f"ps{m}", [128, NCH], mybir.dt.float32))
              for m in range(NM)]
        ksem = ctx.enter_context(nc.semaphore("ksem"))
        xsem = ctx.enter_context(nc.semaphore("xsem"))
        msem = ctx.enter_context(nc.semaphore("msem"))
        vsem = ctx.enter_context(nc.semaphore("vsem"))
        osem = ctx.enter_context(nc.semaphore("osem"))
        block = ctx.enter_context(nc.Block())

        # kp loads on the SP HWDGE queue, xs loads on the ACT queue.
        # pair 0 is split (chunks 0-1 / 2-4) so the PE starts earlier.
        nA_k, nA_x = 2 * NCH, 2 * NV

        @block.sync
        def _(sync):
            sync.dma_start(out=kpt[0][:, :nA_k], in_=kp_d[0, :, :nA_k]).then_inc(ksem, 16)
            sync.dma_start(out=kpt[0][:, nA_k:], in_=kp_d[0, :, nA_k:]).then_inc(ksem, 16)
            for p in range(1, PAIRS):
                sync.dma_start(out=kpt[p][:, :], in_=kp_d[p, :, :]).then_inc(ksem, 16)

        @block.scalar
        def _(scalar):
            scalar.dma_start(out=xst[0][:, :nA_x], in_=xs_d[0, :, :nA_x]).then_inc(xsem, 16)
            scalar.dma_start(out=xst[0][:, nA_x:], in_=xs_d[0, :, nA_x:]).then_inc(xsem, 16)
            for p in range(1, PAIRS):
                scalar.dma_start(out=xst[p][:, :], in_=xs_d[p, :, :]).then_inc(xsem, 16)

        @block.tensor
        def _(tensor):
            def mm(p, c, m, start, stop):
                return tensor.matmul(
                    ps[m][:, :],
                    xst[p][:, c * NV + m * 128:c * NV + (m + 1) * 128],
                    kpt[p][:, c * NCH:(c + 1) * NCH],
                    start=start, stop=stop)

            for p in range(PAIRS):
                if p == 0:
                    tensor.wait_ge(ksem, 16)
                    tensor.wait_ge(xsem, 16)
                    for c in range(2):
                        for m in range(NM):
                            mm(p, c, m, start=(c == 0), stop=False)
                    tensor.wait_ge(ksem, 32)
                    tensor.wait_ge(xsem, 32)
                    for c in range(2, NCC):
                        for m in range(NM):
                            mm(p, c, m, start=False, stop=False)
                elif p < PAIRS - 1:
                    tensor.wait_ge(ksem, 16 * (p + 1) + 16)
                    tensor.wait_ge(xsem, 16 * (p + 1) + 16)
                    for c in range(NCC):
                        for m in range(NM):
                            mm(p, c, m, start=False, stop=False)
                else:
                    tensor.wait_ge(ksem, 16 * PAIRS + 16)
                    tensor.wait_ge(xsem, 16 * PAIRS + 16)
                    # m outer so psum banks complete (and drain) in order
                    for m in range(NM):
                        for c in range(NCC):
                            i = mm(p, c, m, start=False, stop=(c == NCC - 1))
                            if c == NCC - 1:
                                i.then_inc(msem, 1)

        @block.vector
        def _(vector):
            for m in range(NM):
                vector.wait_ge(msem, m + 1)
                vector.tensor_copy(ot[:, m * NCH:(m + 1) * NCH], ps[m][:, :]).then_inc(vsem, 1)

        @block.gpsimd
        def _(gpsimd):
            for m in range(NM):
                gpsimd.wait_ge(vsem, m + 1)
                gpsimd.dma_start(out=out_d[:, m * NCH:(m + 1) * NCH],
                                 in_=ot[:, m * NCH:(m + 1) * NCH]).then_inc(osem, 16)
            gpsimd.wait_ge(osem, 16 * NM)

    _PROGRAM = nc
    return nc


def kernel(x, q_in, q_out, w_ss, w_vs, w_sv, w_vv0, w_vv1, bias):
    global LAST
    from concourse.bass_utils import run_bass_kernel_spmd

    kern = _assemble_kern(np.asarray(q_in, np.float32), np.asarray(q_out, np.float32),
                          np.asarray(w_ss, np.float32), np.asarray(w_vs, np.float32),
                          np.asarray(w_sv, np.float32), np.asarray(w_vv0, np.float32),
                          np.asarray(w_vv1, np.float32))
    xr = np.asarray(x, np.float32).reshape(NCH, P, P, P)
    x_pad = np.zeros((NCH, P + 4, P + 4, P + 4), np.float32)
    x_pad[:, 2:10, 2:10, 2:10] = xr

    # Shifted input per tap (+1 zero slab for padding slots), fp16.
    xsh = np.zeros((K3 + 1, NCH, NV), np.float16)
    t = 0
    for dz in range(K):
        for dy in range(K):
            for dx in range(K):
                xsh[t] = x_pad[:, dz:dz + 8, dy:dy + 8, dx:dx + 8].reshape(NCH, NV)
                t += 1
    kerT = np.zeros((K3 + 1, NCH, NCH), np.float16)          # (tap, in, out)
    kerT[:K3] = kern.transpose(2, 1, 0)

    in_maps = []
    for c in range(NCORES):
        taps = list(range(c, K3, NCORES)) + [K3] * (2 * PAIRS)  # pad w/ zero slab
        taps = taps[:2 * PAIRS]
        kp_c = np.empty((PAIRS, 128, NCC * NCH), np.float16)
        xs_c = np.empty((PAIRS, 128, NCC * NV), np.float16)
        for p in range(PAIRS):
            tA, tB = taps[2 * p], taps[2 * p + 1]
            kb = np.concatenate([kerT[tA], kerT[tB]], axis=0)    # (640, 320)
            xb = np.concatenate([xsh[tA], xsh[tB]], axis=0)      # (640, 512)
            kp_c[p] = kb.reshape(NCC, 128, NCH).transpose(1, 0, 2).reshape(128, NCC * NCH)
            xs_c[p] = xb.reshape(NCC, 128, NV).transpose(1, 0, 2).reshape(128, NCC * NV)
        in_maps.append({"kp": kp_c, "xs": xs_c})

    nc = _build_program()
    res = run_bass_kernel_spmd(nc, in_maps, list(range(NCORES)))
    LAST = res

    acc = np.zeros((128, NM * NCH), np.float64)
    for c in range(NCORES):
        acc += res.results[c]["out_part"]                    # (128, 4*320)
    out = acc.astype(np.float32).reshape(128, NM, NCH).transpose(1, 0, 2).reshape(NV, NCH)
    out = np.ascontiguousarray(out.T).reshape(1, DIM, Q, P, P, P)
    out[:, :C0] += np.asarray(bias, np.float32).reshape(1, C0, 1, 1, 1, 1)
    return out


# revision 8
# speedup vs baseline: 1.2723x; 1.0534x over previous
"""Equivariant PQ-layer conv kernel for 8x TRN2 NeuronCores.

The layer is a 3D conv (SAME, 5^3 taps) with an assembled (320, 320, 125)
kernel over a (320, 8^3) input. The host assembles the conv kernel (cheap)
and shards the 125 taps across the 8 cores (16 tap slots per core, as 8
tap-pairs whose 640 contraction rows split into 5 exact 128-chunks).

Matmul arrangement ("transposed" vs the v1 baseline): PSUM partitions carry
voxels (512 = 4x128 chunks, no partition waste) and the free dim carries all
320 output channels (N=320 <= 512). Per pair: 5 K-chunks x 4 vox-chunks =
20 matmuls of N=320 -> 51200 charged PE rows/core vs 61440 for the
out-channels-on-partitions form (PSUM M=64 waste). lhsT (stationary) is the
shifted-input chunk [128 contraction rows, 128 voxels]; rhs (moving) is the
kernel chunk [128 contraction rows, 320 out channels].

The host pre-shifts x per tap (SPMD program must be identical across cores,
so per-core tap shifts must live in data, not in AP constants).
"""
import numpy as np

C0, C1 = 8, 4
K = 5
G = 8
EPS = 1e-6
R_MAX = 5.5
DIM = C0 + 3 * C1          # 20
Q = 16
P = 8
NCH = DIM * Q              # 320
NV = P * P * P             # 512
K3 = K ** 3                # 125
NCORES = 8
PAIRS = 8                  # tap-pair slots per core: 8*2*8 = 128 >= 125 taps
NCC = 5                    # contraction chunks per pair: 2*320/128
NM = 4                     # vox chunks (psum banks): 512/128

LAST = None                # BassKernelResults of the most recent run
_PROGRAM = None


def _levi_civita():
    e = np.zeros((3, 3, 3), np.float32)
    e[0, 1, 2] = e[1, 2, 0] = e[2, 0, 1] = 1.0
    e[0, 2, 1] = e[2, 1, 0] = e[1, 0, 2] = -1.0
    return e


def _assemble_kern(q_in, q_out, w_ss, w_vs, w_sv, w_vv0, w_vv1):
    """Mirror of the reference kernel assembly, in f32 numpy. -> (320, 320, 125)."""
    offs = np.arange(K, dtype=np.float32) - (K - 1) / 2.0
    oz, oy, ox = np.meshgrid(offs, offs, offs, indexing='ij')
    p_off = np.stack([oz, oy, ox], -1).reshape(-1, 3)
    v = p_off[None, None] - (q_out[:, None, None] - q_in[None, :, None])
    r = np.linalg.norm(v, axis=-1)
    u = np.where(r[..., None] > EPS, v / np.maximum(r, EPS)[..., None], 0.0).astype(np.float32)
    centers = np.linspace(0.0, R_MAX, G).astype(np.float32)
    sigma = R_MAX / (G - 1)
    R = np.exp(-0.5 * ((r[..., None] - centers) / sigma) ** 2).astype(np.float32)
    RY = R[..., None] * u[..., None, :]
    eye3 = np.eye(3, dtype=np.float32)
    eps3 = _levi_civita()
    K_ss = np.einsum('acg,pqkg->apcqk', w_ss, R, optimize=True)
    K_vs = np.einsum('acg,pqkgm->ampcqk', w_vs, RY, optimize=True)
    K_sv = np.einsum('acg,pqkgm->apcmqk', w_sv, RY, optimize=True)
    K_vv = (np.einsum('acg,pqkg,mn->ampcnqk', w_vv0, R, eye3, optimize=True)
            + np.float32(0.7071067811865476) *
            np.einsum('acg,pqkgm,imj->aipcjqk', w_vv1, RY, eps3, optimize=True))
    Qo, Qi = q_out.shape[0], q_in.shape[0]
    top = np.concatenate([K_ss, K_sv.reshape(C0, Qo, 3 * C1, Qi, K3)], axis=2)
    bot = np.concatenate([K_vs.reshape(3 * C1, Qo, C0, Qi, K3),
                          K_vv.reshape(3 * C1, Qo, 3 * C1, Qi, K3)], axis=2)
    kern = np.concatenate([top, bot], axis=0)
    return np.ascontiguousarray(kern.reshape(DIM * Qo, DIM * Qi, K3).astype(np.float32))


def _build_program():
    """SPMD program: 8 tap-pair slots of (kp, xs) -> partial conv output.

    Per pair p and contraction chunk c (5 chunks of 128 rows from the
    640-row tap pair): for vox chunk m: psum[m][128 vox, 320 out] +=
    xs[p][:, c*512+m*128 : +128].T @ kp[p][:, c*320 : +320].
    """
    global _PROGRAM
    if _PROGRAM is not None:
        return _PROGRAM
    from contextlib import ExitStack
    from concourse import bass, mybir

    nc = bass.Bass("TRN2", target_bir_lowering=False, debug=False,
                   enable_asserts=False, num_devices=NCORES)
    kp_d = nc.dram_tensor("kp", [PAIRS, 128, NCC * NCH], mybir.dt.float16,
                          kind="ExternalInput").ap()
    xs_d = nc.dram_tensor("xs", [PAIRS, 128, NCC * NV], mybir.dt.float16,
                          kind="ExternalInput").ap()
    out_d = nc.dram_tensor("out_part", [128, NM * NCH], mybir.dt.float32,
                           kind="ExternalOutput").ap()

    with ExitStack() as ctx:
        kpt = [ctx.enter_context(nc.sbuf_tensor(f"kpt{p}", [128, NCC * NCH], mybir.dt.float16))
               for p in range(PAIRS)]
        xst = [ctx.enter_context(nc.sbuf_tensor(f"xst{p}", [128, NCC * NV], mybir.dt.float16))
               for p in range(PAIRS)]
        ot = ctx.enter_context(nc.sbuf_tensor("ot", [128, NM * NCH], mybir.dt.float32))
        ps = [ctx.enter_context(nc.psum_tensor(f"ps{m}", [128, NCH], mybir.dt.float32))
              for m in range(NM)]
        ksem = ctx.enter_context(nc.semaphore("ksem"))
        xsem = ctx.enter_context(nc.semaphore("xsem"))
        msem = ctx.enter_context(nc.semaphore("msem"))
        vsem = ctx.enter_context(nc.semaphore("vsem"))
        osem = ctx.enter_context(nc.semaphore("osem"))
        block = ctx.enter_context(nc.Block())

        # kp loads on the SP HWDGE queue, xs loads on the ACT queue.
        # pair 0 is split (chunks 0-1 / 2-4) so the PE starts earlier.
        nA_k, nA_x = 2 * NCH, 2 * NV

        @block.sync
        def _(sync):
            sync.dma_start(out=kpt[0][:, :nA_k], in_=kp_d[0, :, :nA_k]).then_inc(ksem, 16)
            sync.dma_start(out=kpt[0][:, nA_k:], in_=kp_d[0, :, nA_k:]).then_inc(ksem, 16)
            for p in range(1, PAIRS):
                sync.dma_start(out=kpt[p][:, :], in_=kp_d[p, :, :]).then_inc(ksem, 16)
            # output drains: SP HWDGE queue is idle once loads are issued.
            # Codegen requires sync info on every DMA, but nothing waits on
            # osem, so the program ends at the last sem propagation.
            for m in range(NM):
                sync.wait_ge(vsem, m + 1)
                sync.dma_start(out=out_d[:, m * NCH:(m + 1) * NCH],
                               in_=ot[:, m * NCH:(m + 1) * NCH]).then_inc(osem, 16)

        @block.scalar
        def _(scalar):
            scalar.dma_start(out=xst[0][:, :nA_x], in_=xs_d[0, :, :nA_x]).then_inc(xsem, 16)
            scalar.dma_start(out=xst[0][:, nA_x:], in_=xs_d[0, :, nA_x:]).then_inc(xsem, 16)
            for p in range(1, PAIRS):
                scalar.dma_start(out=xst[p][:, :], in_=xs_d[p, :, :]).then_inc(xsem, 16)

        @block.tensor
        def _(tensor):
            def mm(p, c, m, start, stop):
                return tensor.matmul(
                    ps[m][:, :],
                    xst[p][:, c * NV + m * 128:c * NV + (m + 1) * 128],
                    kpt[p][:, c * NCH:(c + 1) * NCH],
                    start=start, stop=stop)

            for p in range(PAIRS):
                if p == 0:
                    tensor.wait_ge(ksem, 16)
                    tensor.wait_ge(xsem, 16)
                    for c in range(2):
                        for m in range(NM):
                            mm(p, c, m, start=(c == 0), stop=False)
                    tensor.wait_ge(ksem, 32)
                    tensor.wait_ge(xsem, 32)
                    for c in range(2, NCC):
                        for m in range(NM):
                            mm(p, c, m, start=False, stop=False)
                elif p < PAIRS - 1:
                    tensor.wait_ge(ksem, 16 * (p + 1) + 16)
                    tensor.wait_ge(xsem, 16 * (p + 1) + 16)
                    for c in range(NCC):
                        for m in range(NM):
                            mm(p, c, m, start=False, stop=False)
                else:
                    tensor.wait_ge(ksem, 16 * PAIRS + 16)
                    tensor.wait_ge(xsem, 16 * PAIRS + 16)
                    # m outer so psum banks complete (and drain) in order
                    for m in range(NM):
                        for c in range(NCC):
                            i = mm(p, c, m, start=False, stop=(c == NCC - 1))
                            if c == NCC - 1:
                                i.then_inc(msem, 1)

        @block.vector
        def _(vector):
            for m in range(NM):
                vector.wait_ge(msem, m + 1)
                vector.tensor_copy(ot[:, m * NCH:(m + 1) * NCH], ps[m][:, :]).then_inc(vsem, 1)

    _PROGRAM = nc
    return nc


def kernel(x, q_in, q_out, w_ss, w_vs, w_sv, w_vv0, w_vv1, bias):
    global LAST
    from concourse.bass_utils import run_bass_kernel_spmd

    kern = _assemble_kern(np.asarray(q_in, np.float32), np.asarray(q_out, np.float32),
                          np.asarray(w_ss, np.float32), np.asarray(w_vs, np.float32),
                          np.asarray(w_sv, np.float32), np.asarray(w_vv0, np.float32),
                          np.asarray(w_vv1, np.float32))
    xr = np.asarray(x, np.float32).reshape(NCH, P, P, P)
    x_pad = np.zeros((NCH, P + 4, P + 4, P + 4), np.float32)
    x_pad[:, 2:10, 2:10, 2:10] = xr

    # Shifted input per tap (+1 zero slab for padding slots), fp16.
    xsh = np.zeros((K3 + 1, NCH, NV), np.float16)
    t = 0
    for dz in range(K):
        for dy in range(K):
            for dx in range(K):
                xsh[t] = x_pad[:, dz:dz + 8, dy:dy + 8, dx:dx + 8].reshape(NCH, NV)
                t += 1
    kerT = np.zeros((K3 + 1, NCH, NCH), np.float16)          # (tap, in, out)
    kerT[:K3] = kern.transpose(2, 1, 0)

    in_maps = []
    for c in range(NCORES):
        taps = list(range(c, K3, NCORES)) + [K3] * (2 * PAIRS)  # pad w/ zero slab
        taps = taps[:2 * PAIRS]
        kp_c = np.empty((PAIRS, 128, NCC * NCH), np.float16)
        xs_c = np.empty((PAIRS, 128, NCC * NV), np.float16)
        for p in range(PAIRS):
            tA, tB = taps[2 * p], taps[2 * p + 1]
            kb = np.concatenate([kerT[tA], kerT[tB]], axis=0)    # (640, 320)
            xb = np.concatenate([xsh[tA], xsh[tB]], axis=0)      # (640, 512)
            kp_c[p] = kb.reshape(NCC, 128, NCH).transpose(1, 0, 2).reshape(128, NCC * NCH)
            xs_c[p] = xb.reshape(NCC, 128, NV).transpose(1, 0, 2).reshape(128, NCC * NV)
        in_maps.append({"kp": kp_c, "xs": xs_c})

    nc = _build_program()
    res = run_bass_kernel_spmd(nc, in_maps, list(range(NCORES)))
    LAST = res

    acc = np.zeros((128, NM * NCH), np.float64)
    for c in range(NCORES):
        acc += res.results[c]["out_part"]                    # (128, 4*320)
    out = acc.astype(np.float32).reshape(128, NM, NCH).transpose(1, 0, 2).reshape(NV, NCH)
    out = np.ascontiguousarray(out.T).reshape(1, DIM, Q, P, P, P)
    out[:, :C0] += np.asarray(bias, np.float32).reshape(1, C0, 1, 1, 1, 1)
    return out


# revision 18
# speedup vs baseline: 1.2961x; 1.0187x over previous
"""Equivariant PQ-layer conv kernel for 8x TRN2 NeuronCores.

The layer is a 3D conv (SAME, 5^3 taps) with an assembled (320, 320, 125)
kernel over a (320, 8^3) input. The host assembles the conv kernel (cheap)
and shards the 125 taps across the 8 cores (16 tap slots per core, as 8
tap-pairs whose 640 contraction rows split into 5 exact 128-chunks).

Matmul arrangement ("transposed" vs the v1 baseline): PSUM partitions carry
voxels (512 = 4x128 chunks, no partition waste) and the free dim carries all
320 output channels (N=320 <= 512). Per pair: 5 K-chunks x 4 vox-chunks =
20 matmuls of N=320 -> 51200 charged PE rows/core vs 61440 for the
out-channels-on-partitions form (PSUM M=64 waste). lhsT (stationary) is the
shifted-input chunk [128 contraction rows, 128 voxels]; rhs (moving) is the
kernel chunk [128 contraction rows, 320 out channels].

The host pre-shifts x per tap (SPMD program must be identical across cores,
so per-core tap shifts must live in data, not in AP constants).
"""
import numpy as np

C0, C1 = 8, 4
K = 5
G = 8
EPS = 1e-6
R_MAX = 5.5
DIM = C0 + 3 * C1          # 20
Q = 16
P = 8
NCH = DIM * Q              # 320
NV = P * P * P             # 512
K3 = K ** 3                # 125
NCORES = 8
PAIRS = 8                  # tap-pair slots per core: 8*2*8 = 128 >= 125 taps
NCC = 5                    # contraction chunks per pair: 2*320/128
NM = 4                     # vox chunks (psum banks): 512/128

LAST = None                # BassKernelResults of the most recent run
_PROGRAM = None

# PE p-state management: the TRN2 tensor engine runs at 1.2GHz until it has
# been continuously busy for 3us, then 2.4GHz; any idle gap resets the ramp.
# Warmup fills (tiny N=8 junk matmuls, ~3-7ns each) keep the engine busy from
# ~t=0.2us through the DMA-paced stream so real matmuls run at full clock.
WARMUP = 720             # fills before pair 0's data lands
PAIR_FILLS = [0, 0, 0, 0, 0, 0, 0, 0]  # fills before each pair's wait


def _levi_civita():
    e = np.zeros((3, 3, 3), np.float32)
    e[0, 1, 2] = e[1, 2, 0] = e[2, 0, 1] = 1.0
    e[0, 2, 1] = e[2, 1, 0] = e[1, 0, 2] = -1.0
    return e


def _assemble_kern(q_in, q_out, w_ss, w_vs, w_sv, w_vv0, w_vv1):
    """Mirror of the reference kernel assembly, in f32 numpy. -> (320, 320, 125)."""
    offs = np.arange(K, dtype=np.float32) - (K - 1) / 2.0
    oz, oy, ox = np.meshgrid(offs, offs, offs, indexing='ij')
    p_off = np.stack([oz, oy, ox], -1).reshape(-1, 3)
    v = p_off[None, None] - (q_out[:, None, None] - q_in[None, :, None])
    r = np.linalg.norm(v, axis=-1)
    u = np.where(r[..., None] > EPS, v / np.maximum(r, EPS)[..., None], 0.0).astype(np.float32)
    centers = np.linspace(0.0, R_MAX, G).astype(np.float32)
    sigma = R_MAX / (G - 1)
    R = np.exp(-0.5 * ((r[..., None] - centers) / sigma) ** 2).astype(np.float32)
    RY = R[..., None] * u[..., None, :]
    eye3 = np.eye(3, dtype=np.float32)
    eps3 = _levi_civita()
    K_ss = np.einsum('acg,pqkg->apcqk', w_ss, R, optimize=True)
    K_vs = np.einsum('acg,pqkgm->ampcqk', w_vs, RY, optimize=True)
    K_sv = np.einsum('acg,pqkgm->apcmqk', w_sv, RY, optimize=True)
    K_vv = (np.einsum('acg,pqkg,mn->ampcnqk', w_vv0, R, eye3, optimize=True)
            + np.float32(0.7071067811865476) *
            np.einsum('acg,pqkgm,imj->aipcjqk', w_vv1, RY, eps3, optimize=True))
    Qo, Qi = q_out.shape[0], q_in.shape[0]
    top = np.concatenate([K_ss, K_sv.reshape(C0, Qo, 3 * C1, Qi, K3)], axis=2)
    bot = np.concatenate([K_vs.reshape(3 * C1, Qo, C0, Qi, K3),
                          K_vv.reshape(3 * C1, Qo, 3 * C1, Qi, K3)], axis=2)
    kern = np.concatenate([top, bot], axis=0)
    return np.ascontiguousarray(kern.reshape(DIM * Qo, DIM * Qi, K3).astype(np.float32))


def _build_program():
    """SPMD program: 8 tap-pair slots of (kp, xs) -> partial conv output.

    Per pair p and contraction chunk c (5 chunks of 128 rows from the
    640-row tap pair): for vox chunk m: psum[m][128 vox, 320 out] +=
    xs[p][:, c*512+m*128 : +128].T @ kp[p][:, c*320 : +320].
    """
    global _PROGRAM
    if _PROGRAM is not None:
        return _PROGRAM
    from contextlib import ExitStack
    from concourse import bass, mybir

    nc = bass.Bass("TRN2", target_bir_lowering=False, debug=False,
                   enable_asserts=False, num_devices=NCORES)
    kp_d = nc.dram_tensor("kp", [PAIRS, 128, NCC * NCH], mybir.dt.float16,
                          kind="ExternalInput").ap()
    xs_d = nc.dram_tensor("xs", [PAIRS, 128, NCC * NV], mybir.dt.float16,
                          kind="ExternalInput").ap()
    out_d = nc.dram_tensor("out_part", [128, NM * NCH], mybir.dt.float16,
                           kind="ExternalOutput").ap()

    with ExitStack() as ctx:
        kpt = [ctx.enter_context(nc.sbuf_tensor(f"kpt{p}", [128, NCC * NCH], mybir.dt.float16))
               for p in range(PAIRS)]
        xst = [ctx.enter_context(nc.sbuf_tensor(f"xst{p}", [128, NCC * NV], mybir.dt.float16))
               for p in range(PAIRS)]
        ot = ctx.enter_context(nc.sbuf_tensor("ot", [128, NM * NCH], mybir.dt.float16))
        junk = ctx.enter_context(nc.sbuf_tensor("junk", [128, 8], mybir.dt.float16))
        ps = [ctx.enter_context(nc.psum_tensor(f"ps{m}", [128, NCH], mybir.dt.float32))
              for m in range(NM)]
        psj = ctx.enter_context(nc.psum_tensor("psj", [8, 8], mybir.dt.float32))
        ksem = ctx.enter_context(nc.semaphore("ksem"))
        xsem = ctx.enter_context(nc.semaphore("xsem"))
        msem = ctx.enter_context(nc.semaphore("msem"))
        vsem = ctx.enter_context(nc.semaphore("vsem"))
        osem = ctx.enter_context(nc.semaphore("osem"))
        jsem = ctx.enter_context(nc.semaphore("jsem"))
        wsem = ctx.enter_context(nc.semaphore("wsem"))
        block = ctx.enter_context(nc.Block())

        # kp loads on the SP HWDGE queue, xs loads on the ACT queue.
        # pair 0 is split (chunks 0-1 / 2-4) so the PE starts earlier.
        nA_k, nA_x = 2 * NCH, 2 * NV

        @block.sync
        def _(sync):
            sync.dma_start(out=kpt[0][:, :nA_k], in_=kp_d[0, :, :nA_k]).then_inc(ksem, 16)
            sync.dma_start(out=kpt[0][:, nA_k:], in_=kp_d[0, :, nA_k:]).then_inc(ksem, 16)
            for p in range(1, PAIRS):
                sync.dma_start(out=kpt[p][:, :], in_=kp_d[p, :, :]).then_inc(ksem, 16)
            # output drains split across queues: SP issues banks 0/2 (after
            # DVE copies), Act issues banks 1/3 (after its own copies).
            # Nothing waits on osem; the program ends at the last sem prop.
            for i, m in enumerate((0, 2)):
                sync.wait_ge(vsem, i + 1)
                sync.dma_start(out=out_d[:, m * NCH:(m + 1) * NCH],
                               in_=ot[:, m * NCH:(m + 1) * NCH]).then_inc(osem, 16)

        @block.scalar
        def _(scalar):
            scalar.dma_start(out=xst[0][:, :nA_x], in_=xs_d[0, :, :nA_x]).then_inc(xsem, 16)
            scalar.dma_start(out=xst[0][:, nA_x:], in_=xs_d[0, :, nA_x:]).then_inc(xsem, 16)
            for p in range(1, PAIRS):
                scalar.dma_start(out=xst[p][:, :], in_=xs_d[p, :, :]).then_inc(xsem, 16)
            for i, m in enumerate((1, 3)):
                scalar.wait_ge(msem, m + 1)
                scalar.copy(ot[:, m * NCH:(m + 1) * NCH], ps[m][:, :]).then_inc(wsem, 1)
                scalar.wait_ge(wsem, i + 1)
                scalar.dma_start(out=out_d[:, m * NCH:(m + 1) * NCH],
                                 in_=ot[:, m * NCH:(m + 1) * NCH]).then_inc(osem, 16)

        @block.tensor
        def _(tensor):
            def mm(p, c, m, start, stop):
                return tensor.matmul(
                    ps[m][:, :],
                    xst[p][:, c * NV + m * 128:c * NV + (m + 1) * 128],
                    kpt[p][:, c * NCH:(c + 1) * NCH],
                    start=start, stop=stop)

            def fill(n):
                for _ in range(n):
                    tensor.matmul(psj[:, :], junk[:, :], junk[:, :],
                                  start=True, stop=True)

            tensor.wait_ge(jsem, 1)
            fill(WARMUP)
            for p in range(PAIRS):
                fill(PAIR_FILLS[p])
                if p == 0:
                    tensor.wait_ge(ksem, 16)
                    tensor.wait_ge(xsem, 16)
                    for c in range(2):
                        for m in range(NM):
                            mm(p, c, m, start=(c == 0), stop=False)
                    tensor.wait_ge(ksem, 32)
                    tensor.wait_ge(xsem, 32)
                    for c in range(2, NCC):
                        for m in range(NM):
                            mm(p, c, m, start=False, stop=False)
                elif p < PAIRS - 1:
                    tensor.wait_ge(ksem, 16 * (p + 1) + 16)
                    tensor.wait_ge(xsem, 16 * (p + 1) + 16)
                    for c in range(NCC):
                        for m in range(NM):
                            mm(p, c, m, start=False, stop=False)
                else:
                    tensor.wait_ge(ksem, 16 * PAIRS + 16)
                    tensor.wait_ge(xsem, 16 * PAIRS + 16)
                    # m outer so psum banks complete (and drain) in order
                    for m in range(NM):
                        for c in range(NCC):
                            i = mm(p, c, m, start=False, stop=(c == NCC - 1))
                            if c == NCC - 1:
                                i.then_inc(msem, 1)

        @block.vector
        def _(vector):
            vector.memset(junk[:, :], 0.0).then_inc(jsem, 1)
            for m in (0, 2):
                vector.wait_ge(msem, m + 1)
                vector.tensor_copy(ot[:, m * NCH:(m + 1) * NCH], ps[m][:, :]).then_inc(vsem, 1)

    _PROGRAM = nc
    return nc


def kernel(x, q_in, q_out, w_ss, w_vs, w_sv, w_vv0, w_vv1, bias):
    global LAST
    from concourse.bass_utils import run_bass_kernel_spmd

    kern = _assemble_kern(np.asarray(q_in, np.float32), np.asarray(q_out, np.float32),
                          np.asarray(w_ss, np.float32), np.asarray(w_vs, np.float32),
                          np.asarray(w_sv, np.float32), np.asarray(w_vv0, np.float32),
                          np.asarray(w_vv1, np.float32))
    xr = np.asarray(x, np.float32).reshape(NCH, P, P, P)
    x_pad = np.zeros((NCH, P + 4, P + 4, P + 4), np.float32)
    x_pad[:, 2:10, 2:10, 2:10] = xr

    # Shifted input per tap (+1 zero slab for padding slots), fp16.
    xsh = np.zeros((K3 + 1, NCH, NV), np.float16)
    t = 0
    for dz in range(K):
        for dy in range(K):
            for dx in range(K):
                xsh[t] = x_pad[:, dz:dz + 8, dy:dy + 8, dx:dx + 8].reshape(NCH, NV)
                t += 1
    kerT = np.zeros((K3 + 1, NCH, NCH), np.float16)          # (tap, in, out)
    kerT[:K3] = kern.transpose(2, 1, 0)

    in_maps = []
    for c in range(NCORES):
        taps = list(range(c, K3, NCORES)) + [K3] * (2 * PAIRS)  # pad w/ zero slab
        taps = taps[:2 * PAIRS]
        kp_c = np.empty((PAIRS, 128, NCC * NCH), np.float16)
        xs_c = np.empty((PAIRS, 128, NCC * NV), np.float16)
        for p in range(PAIRS):
            tA, tB = taps[2 * p], taps[2 * p + 1]
            kb = np.concatenate([kerT[tA], kerT[tB]], axis=0)    # (640, 320)
            xb = np.concatenate([xsh[tA], xsh[tB]], axis=0)      # (640, 512)
            kp_c[p] = kb.reshape(NCC, 128, NCH).transpose(1, 0, 2).reshape(128, NCC * NCH)
            xs_c[p] = xb.reshape(NCC, 128, NV).transpose(1, 0, 2).reshape(128, NCC * NV)
        in_maps.append({"kp": kp_c, "xs": xs_c})

    nc = _build_program()
    res = run_bass_kernel_spmd(nc, in_maps, list(range(NCORES)))
    LAST = res

    acc = np.zeros((128, NM * NCH), np.float64)
    for c in range(NCORES):
        acc += res.results[c]["out_part"]                    # (128, 4*320)
    out = acc.astype(np.float32).reshape(128, NM, NCH).transpose(1, 0, 2).reshape(NV, NCH)
    out = np.ascontiguousarray(out.T).reshape(1, DIM, Q, P, P, P)
    out[:, :C0] += np.asarray(bias, np.float32).reshape(1, C0, 1, 1, 1, 1)
    return out


# revision 19
# speedup vs baseline: 1.3083x; 1.0094x over previous
"""Equivariant PQ-layer conv kernel for 8x TRN2 NeuronCores.

The layer is a 3D conv (SAME, 5^3 taps) with an assembled (320, 320, 125)
kernel over a (320, 8^3) input. The host assembles the conv kernel (cheap)
and shards the 125 taps across the 8 cores (16 tap slots per core, as 8
tap-pairs whose 640 contraction rows split into 5 exact 128-chunks).

Matmul arrangement ("transposed" vs the v1 baseline): PSUM partitions carry
voxels (512 = 4x128 chunks, no partition waste) and the free dim carries all
320 output channels (N=320 <= 512). Per pair: 5 K-chunks x 4 vox-chunks =
20 matmuls of N=320 -> 51200 charged PE rows/core vs 61440 for the
out-channels-on-partitions form (PSUM M=64 waste). lhsT (stationary) is the
shifted-input chunk [128 contraction rows, 128 voxels]; rhs (moving) is the
kernel chunk [128 contraction rows, 320 out channels].

The host pre-shifts x per tap (SPMD program must be identical across cores,
so per-core tap shifts must live in data, not in AP constants).
"""
import numpy as np

C0, C1 = 8, 4
K = 5
G = 8
EPS = 1e-6
R_MAX = 5.5
DIM = C0 + 3 * C1          # 20
Q = 16
P = 8
NCH = DIM * Q              # 320
NV = P * P * P             # 512
K3 = K ** 3                # 125
NCORES = 8
PAIRS = 8                  # tap-pair slots per core: 8*2*8 = 128 >= 125 taps
NCC = 5                    # contraction chunks per pair: 2*320/128
NM = 4                     # vox chunks (psum banks): 512/128

LAST = None                # BassKernelResults of the most recent run
_PROGRAM = None

# PE p-state management: the TRN2 tensor engine runs at 1.2GHz until it has
# been continuously busy for 3us, then 2.4GHz; any idle gap resets the ramp.
# Warmup fills (tiny N=8 junk matmuls, ~3-7ns each) keep the engine busy from
# ~t=0.2us through the DMA-paced stream so real matmuls run at full clock.
WARMUP = 1100            # fills before pair 0's data lands
PAIR_FILLS = [0, 0, 0, 0, 0, 0, 0, 0]  # fills before each pair's wait


def _levi_civita():
    e = np.zeros((3, 3, 3), np.float32)
    e[0, 1, 2] = e[1, 2, 0] = e[2, 0, 1] = 1.0
    e[0, 2, 1] = e[2, 1, 0] = e[1, 0, 2] = -1.0
    return e


def _assemble_kern(q_in, q_out, w_ss, w_vs, w_sv, w_vv0, w_vv1):
    """Mirror of the reference kernel assembly, in f32 numpy. -> (320, 320, 125)."""
    offs = np.arange(K, dtype=np.float32) - (K - 1) / 2.0
    oz, oy, ox = np.meshgrid(offs, offs, offs, indexing='ij')
    p_off = np.stack([oz, oy, ox], -1).reshape(-1, 3)
    v = p_off[None, None] - (q_out[:, None, None] - q_in[None, :, None])
    r = np.linalg.norm(v, axis=-1)
    u = np.where(r[..., None] > EPS, v / np.maximum(r, EPS)[..., None], 0.0).astype(np.float32)
    centers = np.linspace(0.0, R_MAX, G).astype(np.float32)
    sigma = R_MAX / (G - 1)
    R = np.exp(-0.5 * ((r[..., None] - centers) / sigma) ** 2).astype(np.float32)
    RY = R[..., None] * u[..., None, :]
    eye3 = np.eye(3, dtype=np.float32)
    eps3 = _levi_civita()
    K_ss = np.einsum('acg,pqkg->apcqk', w_ss, R, optimize=True)
    K_vs = np.einsum('acg,pqkgm->ampcqk', w_vs, RY, optimize=True)
    K_sv = np.einsum('acg,pqkgm->apcmqk', w_sv, RY, optimize=True)
    K_vv = (np.einsum('acg,pqkg,mn->ampcnqk', w_vv0, R, eye3, optimize=True)
            + np.float32(0.7071067811865476) *
            np.einsum('acg,pqkgm,imj->aipcjqk', w_vv1, RY, eps3, optimize=True))
    Qo, Qi = q_out.shape[0], q_in.shape[0]
    top = np.concatenate([K_ss, K_sv.reshape(C0, Qo, 3 * C1, Qi, K3)], axis=2)
    bot = np.concatenate([K_vs.reshape(3 * C1, Qo, C0, Qi, K3),
                          K_vv.reshape(3 * C1, Qo, 3 * C1, Qi, K3)], axis=2)
    kern = np.concatenate([top, bot], axis=0)
    return np.ascontiguousarray(kern.reshape(DIM * Qo, DIM * Qi, K3).astype(np.float32))


def _build_program():
    """SPMD program: 8 tap-pair slots of (kp, xs) -> partial conv output.

    Per pair p and contraction chunk c (5 chunks of 128 rows from the
    640-row tap pair): for vox chunk m: psum[m][128 vox, 320 out] +=
    xs[p][:, c*512+m*128 : +128].T @ kp[p][:, c*320 : +320].
    """
    global _PROGRAM
    if _PROGRAM is not None:
        return _PROGRAM
    from contextlib import ExitStack
    from concourse import bass, mybir

    nc = bass.Bass("TRN2", target_bir_lowering=False, debug=False,
                   enable_asserts=False, num_devices=NCORES)
    kp_d = nc.dram_tensor("kp", [PAIRS, 128, NCC * NCH], mybir.dt.float16,
                          kind="ExternalInput").ap()
    xs_d = nc.dram_tensor("xs", [PAIRS, 128, NCC * NV], mybir.dt.float16,
                          kind="ExternalInput").ap()
    out_d = nc.dram_tensor("out_part", [128, NM * NCH], mybir.dt.float16,
                           kind="ExternalOutput").ap()

    with ExitStack() as ctx:
        kpt = [ctx.enter_context(nc.sbuf_tensor(f"kpt{p}", [128, NCC * NCH], mybir.dt.float16))
               for p in range(PAIRS)]
        xst = [ctx.enter_context(nc.sbuf_tensor(f"xst{p}", [128, NCC * NV], mybir.dt.float16))
               for p in range(PAIRS)]
        ot = ctx.enter_context(nc.sbuf_tensor("ot", [128, NM * NCH], mybir.dt.float16))
        junk = ctx.enter_context(nc.sbuf_tensor("junk", [128, 8], mybir.dt.float16))
        ps = [ctx.enter_context(nc.psum_tensor(f"ps{m}", [128, NCH], mybir.dt.float32))
              for m in range(NM)]
        psj = ctx.enter_context(nc.psum_tensor("psj", [8, 8], mybir.dt.float32))
        ksem = ctx.enter_context(nc.semaphore("ksem"))
        xsem = ctx.enter_context(nc.semaphore("xsem"))
        msem = ctx.enter_context(nc.semaphore("msem"))
        vsem = ctx.enter_context(nc.semaphore("vsem"))
        osem = ctx.enter_context(nc.semaphore("osem"))
        jsem = ctx.enter_context(nc.semaphore("jsem"))
        wsem = ctx.enter_context(nc.semaphore("wsem"))
        block = ctx.enter_context(nc.Block())

        # kp loads on the SP HWDGE queue, xs loads on the ACT queue. One DMA
        # per pair: fewer, larger transfers keep the DMA device gapless (the
        # whole schedule is stream-end-gated, so pair-0 latency is slack).

        @block.sync
        def _(sync):
            for p in range(PAIRS):
                sync.dma_start(out=kpt[p][:, :], in_=kp_d[p, :, :]).then_inc(ksem, 16)
            # output drains split across queues: SP issues banks 0/2 (after
            # DVE copies), Act issues banks 1/3 (after its own copies).
            # Nothing waits on osem; the program ends at the last sem prop.
            for i, m in enumerate((0, 2)):
                sync.wait_ge(vsem, i + 1)
                sync.dma_start(out=out_d[:, m * NCH:(m + 1) * NCH],
                               in_=ot[:, m * NCH:(m + 1) * NCH]).then_inc(osem, 16)

        @block.scalar
        def _(scalar):
            for p in range(PAIRS):
                scalar.dma_start(out=xst[p][:, :], in_=xs_d[p, :, :]).then_inc(xsem, 16)
            for i, m in enumerate((1, 3)):
                scalar.wait_ge(msem, m + 1)
                scalar.copy(ot[:, m * NCH:(m + 1) * NCH], ps[m][:, :]).then_inc(wsem, 1)
                scalar.wait_ge(wsem, i + 1)
                scalar.dma_start(out=out_d[:, m * NCH:(m + 1) * NCH],
                                 in_=ot[:, m * NCH:(m + 1) * NCH]).then_inc(osem, 16)

        @block.tensor
        def _(tensor):
            def mm(p, c, m, start, stop):
                return tensor.matmul(
                    ps[m][:, :],
                    xst[p][:, c * NV + m * 128:c * NV + (m + 1) * 128],
                    kpt[p][:, c * NCH:(c + 1) * NCH],
                    start=start, stop=stop)

            def fill(n):
                for _ in range(n):
                    tensor.matmul(psj[:, :], junk[:, :], junk[:, :],
                                  start=True, stop=True)

            tensor.wait_ge(jsem, 1)
            fill(WARMUP)
            for p in range(PAIRS):
                fill(PAIR_FILLS[p])
                tensor.wait_ge(ksem, 16 * (p + 1))
                tensor.wait_ge(xsem, 16 * (p + 1))
                if p < PAIRS - 1:
                    for c in range(NCC):
                        for m in range(NM):
                            mm(p, c, m, start=(p == 0 and c == 0), stop=False)
                else:
                    # m outer so psum banks complete (and drain) in order
                    for m in range(NM):
                        for c in range(NCC):
                            i = mm(p, c, m, start=False, stop=(c == NCC - 1))
                            if c == NCC - 1:
                                i.then_inc(msem, 1)

        @block.vector
        def _(vector):
            vector.memset(junk[:, :], 0.0).then_inc(jsem, 1)
            for m in (0, 2):
                vector.wait_ge(msem, m + 1)
                vector.tensor_copy(ot[:, m * NCH:(m + 1) * NCH], ps[m][:, :]).then_inc(vsem, 1)

    _PROGRAM = nc
    return nc


def kernel(x, q_in, q_out, w_ss, w_vs, w_sv, w_vv0, w_vv1, bias):
    global LAST
    from concourse.bass_utils import run_bass_kernel_spmd

    kern = _assemble_kern(np.asarray(q_in, np.float32), np.asarray(q_out, np.float32),
                          np.asarray(w_ss, np.float32), np.asarray(w_vs, np.float32),
                          np.asarray(w_sv, np.float32), np.asarray(w_vv0, np.float32),
                          np.asarray(w_vv1, np.float32))
    xr = np.asarray(x, np.float32).reshape(NCH, P, P, P)
    x_pad = np.zeros((NCH, P + 4, P + 4, P + 4), np.float32)
    x_pad[:, 2:10, 2:10, 2:10] = xr

    # Shifted input per tap (+1 zero slab for padding slots), fp16.
    xsh = np.zeros((K3 + 1, NCH, NV), np.float16)
    t = 0
    for dz in range(K):
        for dy in range(K):
            for dx in range(K):
                xsh[t] = x_pad[:, dz:dz + 8, dy:dy + 8, dx:dx + 8].reshape(NCH, NV)
                t += 1
    kerT = np.zeros((K3 + 1, NCH, NCH), np.float16)          # (tap, in, out)
    kerT[:K3] = kern.transpose(2, 1, 0)

    in_maps = []
    for c in range(NCORES):
        taps = list(range(c, K3, NCORES)) + [K3] * (2 * PAIRS)  # pad w/ zero slab
        taps = taps[:2 * PAIRS]
        kp_c = np.empty((PAIRS, 128, NCC * NCH), np.float16)
        xs_c = np.empty((PAIRS, 128, NCC * NV), np.float16)
        for p in range(PAIRS):
            tA, tB = taps[2 * p], taps[2 * p + 1]
            kb = np.concatenate([kerT[tA], kerT[tB]], axis=0)    # (640, 320)
            xb = np.concatenate([xsh[tA], xsh[tB]], axis=0)      # (640, 512)
            kp_c[p] = kb.reshape(NCC, 128, NCH).transpose(1, 0, 2).reshape(128, NCC * NCH)
            xs_c[p] = xb.reshape(NCC, 128, NV).transpose(1, 0, 2).reshape(128, NCC * NV)
        in_maps.append({"kp": kp_c, "xs": xs_c})

    nc = _build_program()
    res = run_bass_kernel_spmd(nc, in_maps, list(range(NCORES)))
    LAST = res

    acc = np.zeros((128, NM * NCH), np.float64)
    for c in range(NCORES):
        acc += res.results[c]["out_part"]                    # (128, 4*320)
    out = acc.astype(np.float32).reshape(128, NM, NCH).transpose(1, 0, 2).reshape(NV, NCH)
    out = np.ascontiguousarray(out.T).reshape(1, DIM, Q, P, P, P)
    out[:, :C0] += np.asarray(bias, np.float32).reshape(1, C0, 1, 1, 1, 1)
    return out


# revision 20
# speedup vs baseline: 1.3143x; 1.0046x over previous
"""Equivariant PQ-layer conv kernel for 8x TRN2 NeuronCores.

The layer is a 3D conv (SAME, 5^3 taps) with an assembled (320, 320, 125)
kernel over a (320, 8^3) input. The host assembles the conv kernel (cheap)
and shards the 125 taps across the 8 cores (16 tap slots per core, as 8
tap-pairs whose 640 contraction rows split into 5 exact 128-chunks).

Matmul arrangement ("transposed" vs the v1 baseline): PSUM partitions carry
voxels (512 = 4x128 chunks, no partition waste) and the free dim carries all
320 output channels (N=320 <= 512). Per pair: 5 K-chunks x 4 vox-chunks =
20 matmuls of N=320 -> 51200 charged PE rows/core vs 61440 for the
out-channels-on-partitions form (PSUM M=64 waste). lhsT (stationary) is the
shifted-input chunk [128 contraction rows, 128 voxels]; rhs (moving) is the
kernel chunk [128 contraction rows, 320 out channels].

The host pre-shifts x per tap (SPMD program must be identical across cores,
so per-core tap shifts must live in data, not in AP constants).
"""
import numpy as np

C0, C1 = 8, 4
K = 5
G = 8
EPS = 1e-6
R_MAX = 5.5
DIM = C0 + 3 * C1          # 20
Q = 16
P = 8
NCH = DIM * Q              # 320
NV = P * P * P             # 512
K3 = K ** 3                # 125
NCORES = 8
PAIRS = 8                  # tap-pair slots per core: 8*2*8 = 128 >= 125 taps
NCC = 5                    # contraction chunks per pair: 2*320/128
NM = 4                     # vox chunks (psum banks): 512/128

LAST = None                # BassKernelResults of the most recent run
_PROGRAM = None

# PE p-state management: the TRN2 tensor engine runs at 1.2GHz until it has
# been continuously busy for 3us, then 2.4GHz; any idle gap resets the ramp.
# Warmup fills (tiny N=8 junk matmuls, ~3-7ns each) keep the engine busy from
# ~t=0.2us through the DMA-paced stream so real matmuls run at full clock.
WARMUP = 1100            # fills before pair 0's data lands
PAIR_FILLS = [0, 0, 0, 0, 0, 0, 0, 0]  # fills before each pair's wait


def _levi_civita():
    e = np.zeros((3, 3, 3), np.float32)
    e[0, 1, 2] = e[1, 2, 0] = e[2, 0, 1] = 1.0
    e[0, 2, 1] = e[2, 1, 0] = e[1, 0, 2] = -1.0
    return e


def _assemble_kern(q_in, q_out, w_ss, w_vs, w_sv, w_vv0, w_vv1):
    """Mirror of the reference kernel assembly, in f32 numpy. -> (320, 320, 125)."""
    offs = np.arange(K, dtype=np.float32) - (K - 1) / 2.0
    oz, oy, ox = np.meshgrid(offs, offs, offs, indexing='ij')
    p_off = np.stack([oz, oy, ox], -1).reshape(-1, 3)
    v = p_off[None, None] - (q_out[:, None, None] - q_in[None, :, None])
    r = np.linalg.norm(v, axis=-1)
    u = np.where(r[..., None] > EPS, v / np.maximum(r, EPS)[..., None], 0.0).astype(np.float32)
    centers = np.linspace(0.0, R_MAX, G).astype(np.float32)
    sigma = R_MAX / (G - 1)
    R = np.exp(-0.5 * ((r[..., None] - centers) / sigma) ** 2).astype(np.float32)
    RY = R[..., None] * u[..., None, :]
    eye3 = np.eye(3, dtype=np.float32)
    eps3 = _levi_civita()
    K_ss = np.einsum('acg,pqkg->apcqk', w_ss, R, optimize=True)
    K_vs = np.einsum('acg,pqkgm->ampcqk', w_vs, RY, optimize=True)
    K_sv = np.einsum('acg,pqkgm->apcmqk', w_sv, RY, optimize=True)
    K_vv = (np.einsum('acg,pqkg,mn->ampcnqk', w_vv0, R, eye3, optimize=True)
            + np.float32(0.7071067811865476) *
            np.einsum('acg,pqkgm,imj->aipcjqk', w_vv1, RY, eps3, optimize=True))
    Qo, Qi = q_out.shape[0], q_in.shape[0]
    top = np.concatenate([K_ss, K_sv.reshape(C0, Qo, 3 * C1, Qi, K3)], axis=2)
    bot = np.concatenate([K_vs.reshape(3 * C1, Qo, C0, Qi, K3),
                          K_vv.reshape(3 * C1, Qo, 3 * C1, Qi, K3)], axis=2)
    kern = np.concatenate([top, bot], axis=0)
    return np.ascontiguousarray(kern.reshape(DIM * Qo, DIM * Qi, K3).astype(np.float32))


def _build_program():
    """SPMD program: 8 tap-pair slots of (kp, xs) -> partial conv output.

    Per pair p and contraction chunk c (5 chunks of 128 rows from the
    640-row tap pair): for vox chunk m: psum[m][128 vox, 320 out] +=
    xs[p][:, c*512+m*128 : +128].T @ kp[p][:, c*320 : +320].
    """
    global _PROGRAM
    if _PROGRAM is not None:
        return _PROGRAM
    from contextlib import ExitStack
    from concourse import bass, mybir

    nc = bass.Bass("TRN2", target_bir_lowering=False, debug=False,
                   enable_asserts=False, num_devices=NCORES)
    kp_d = nc.dram_tensor("kp", [PAIRS, 128, NCC * NCH], mybir.dt.float16,
                          kind="ExternalInput").ap()
    xs_d = nc.dram_tensor("xs", [PAIRS, 128, NCC * NV], mybir.dt.float16,
                          kind="ExternalInput").ap()
    out_d = nc.dram_tensor("out_part", [128, NM * NCH], mybir.dt.float16,
                           kind="ExternalOutput").ap()

    with ExitStack() as ctx:
        kpt = [ctx.enter_context(nc.sbuf_tensor(f"kpt{p}", [128, NCC * NCH], mybir.dt.float16))
               for p in range(PAIRS)]
        xst = [ctx.enter_context(nc.sbuf_tensor(f"xst{p}", [128, NCC * NV], mybir.dt.float16))
               for p in range(PAIRS)]
        ot = ctx.enter_context(nc.sbuf_tensor("ot", [128, NM * NCH], mybir.dt.float16))
        junk = ctx.enter_context(nc.sbuf_tensor("junk", [128, 8], mybir.dt.float16))
        ps = [ctx.enter_context(nc.psum_tensor(f"ps{m}", [128, NCH], mybir.dt.float32))
              for m in range(NM)]
        psj = ctx.enter_context(nc.psum_tensor("psj", [8, 8], mybir.dt.float32))
        ksem = ctx.enter_context(nc.semaphore("ksem"))
        xsem = ctx.enter_context(nc.semaphore("xsem"))
        msem = ctx.enter_context(nc.semaphore("msem"))
        vsem = ctx.enter_context(nc.semaphore("vsem"))
        osem = ctx.enter_context(nc.semaphore("osem"))
        jsem = ctx.enter_context(nc.semaphore("jsem"))
        wsem = ctx.enter_context(nc.semaphore("wsem"))
        block = ctx.enter_context(nc.Block())

        # kp loads on the SP HWDGE queue, xs loads on the ACT queue. One DMA
        # per pair: fewer, larger transfers keep the DMA device gapless (the
        # whole schedule is stream-end-gated, so pair-0 latency is slack).

        @block.sync
        def _(sync):
            for p in range(PAIRS):
                sync.dma_start(out=kpt[p][:, :], in_=kp_d[p, :, :]).then_inc(ksem, 16)
            # output drains split across queues: SP issues banks 0/2 (after
            # DVE copies), Act issues banks 1/3 (after its own copies).
            # Nothing waits on osem; the program ends at the last sem prop.
            for i, m in enumerate((0, 2)):
                sync.wait_ge(vsem, i + 1)
                sync.dma_start(out=out_d[:, m * NCH:(m + 1) * NCH],
                               in_=ot[:, m * NCH:(m + 1) * NCH]).then_inc(osem, 16)
            sync.wait_ge(wsem, 2)
            sync.dma_start(out=out_d[:, 3 * NCH:4 * NCH],
                           in_=ot[:, 3 * NCH:4 * NCH]).then_inc(osem, 16)

        @block.scalar
        def _(scalar):
            for p in range(PAIRS):
                scalar.dma_start(out=xst[p][:, :], in_=xs_d[p, :, :]).then_inc(xsem, 16)
            scalar.wait_ge(msem, 2)
            scalar.copy(ot[:, NCH:2 * NCH], ps[1][:, :]).then_inc(wsem, 1)
            scalar.wait_ge(wsem, 1)
            scalar.dma_start(out=out_d[:, NCH:2 * NCH],
                             in_=ot[:, NCH:2 * NCH]).then_inc(osem, 16)
            scalar.wait_ge(msem, 4)
            scalar.copy(ot[:, 3 * NCH:4 * NCH], ps[3][:, :]).then_inc(wsem, 1)

        @block.tensor
        def _(tensor):
            def mm(p, c, m, start, stop):
                return tensor.matmul(
                    ps[m][:, :],
                    xst[p][:, c * NV + m * 128:c * NV + (m + 1) * 128],
                    kpt[p][:, c * NCH:(c + 1) * NCH],
                    start=start, stop=stop)

            def fill(n):
                for _ in range(n):
                    tensor.matmul(psj[:, :], junk[:, :], junk[:, :],
                                  start=True, stop=True)

            tensor.wait_ge(jsem, 1)
            fill(WARMUP)
            for p in range(PAIRS):
                fill(PAIR_FILLS[p])
                tensor.wait_ge(ksem, 16 * (p + 1))
                tensor.wait_ge(xsem, 16 * (p + 1))
                if p < PAIRS - 1:
                    for c in range(NCC):
                        for m in range(NM):
                            mm(p, c, m, start=(p == 0 and c == 0), stop=False)
                else:
                    # m outer so psum banks complete (and drain) in order
                    for m in range(NM):
                        for c in range(NCC):
                            i = mm(p, c, m, start=False, stop=(c == NCC - 1))
                            if c == NCC - 1:
                                i.then_inc(msem, 1)

        @block.vector
        def _(vector):
            vector.memset(junk[:, :], 0.0).then_inc(jsem, 1)
            for m in (0, 2):
                vector.wait_ge(msem, m + 1)
                vector.tensor_copy(ot[:, m * NCH:(m + 1) * NCH], ps[m][:, :]).then_inc(vsem, 1)

    _PROGRAM = nc
    return nc


def kernel(x, q_in, q_out, w_ss, w_vs, w_sv, w_vv0, w_vv1, bias):
    global LAST
    from concourse.bass_utils import run_bass_kernel_spmd

    kern = _assemble_kern(np.asarray(q_in, np.float32), np.asarray(q_out, np.float32),
                          np.asarray(w_ss, np.float32), np.asarray(w_vs, np.float32),
                          np.asarray(w_sv, np.float32), np.asarray(w_vv0, np.float32),
                          np.asarray(w_vv1, np.float32))
    xr = np.asarray(x, np.float32).reshape(NCH, P, P, P)
    x_pad = np.zeros((NCH, P + 4, P + 4, P + 4), np.float32)
    x_pad[:, 2:10, 2:10, 2:10] = xr

    # Shifted input per tap (+1 zero slab for padding slots), fp16.
    xsh = np.zeros((K3 + 1, NCH, NV), np.float16)
    t = 0
    for dz in range(K):
        for dy in range(K):
            for dx in range(K):
                xsh[t] = x_pad[:, dz:dz + 8, dy:dy + 8, dx:dx + 8].reshape(NCH, NV)
                t += 1
    kerT = np.zeros((K3 + 1, NCH, NCH), np.float16)          # (tap, in, out)
    kerT[:K3] = kern.transpose(2, 1, 0)

    in_maps = []
    for c in range(NCORES):
        taps = list(range(c, K3, NCORES)) + [K3] * (2 * PAIRS)  # pad w/ zero slab
        taps = taps[:2 * PAIRS]
        kp_c = np.empty((PAIRS, 128, NCC * NCH), np.float16)
        xs_c = np.empty((PAIRS, 128, NCC * NV), np.float16)
        for p in range(PAIRS):
            tA, tB = taps[2 * p], taps[2 * p + 1]
            kb = np.concatenate([kerT[tA], kerT[tB]], axis=0)    # (640, 320)
            xb = np.concatenate([xsh[tA], xsh[tB]], axis=0)      # (640, 512)
            kp_c[p] = kb.reshape(NCC, 128, NCH).transpose(1, 0, 2).reshape(128, NCC * NCH)
            xs_c[p] = xb.reshape(NCC, 128, NV).transpose(1, 0, 2).reshape(128, NCC * NV)
        in_maps.append({"kp": kp_c, "xs": xs_c})

    nc = _build_program()
    res = run_bass_kernel_spmd(nc, in_maps, list(range(NCORES)))
    LAST = res

    acc = np.zeros((128, NM * NCH), np.float64)
    for c in range(NCORES):
        acc += res.results[c]["out_part"]                    # (128, 4*320)
    out = acc.astype(np.float32).reshape(128, NM, NCH).transpose(1, 0, 2).reshape(NV, NCH)
    out = np.ascontiguousarray(out.T).reshape(1, DIM, Q, P, P, P)
    out[:, :C0] += np.asarray(bias, np.float32).reshape(1, C0, 1, 1, 1, 1)
    return out
